# revision 14
# baseline (speedup 1.0000x reference)
"""BiLSTM encoder Bass/Tile kernel for TRN2 — layer-split across core pairs.

Architecture (8 cores, uniform SPMD program, one LSTM chain per core):
 - 4 core pairs: (0,1)=fwd batch 0:16, (2,3)=fwd 16:32, (4,5)=bwd 0:16,
   (6,7)=bwd 16:32. Even core runs layer 0, odd core runs layer 1 lagged by
   2 chunks; the layer-0 hidden-state chunks flow even->odd through an
   AllGather pair exchange (DRAM bounce buffers).
 - Each core runs ONE recurrent chain of T steps at b=16: per step 64
   LDW+MM pairs (N=16) for Wh·h plus one identity matmul that injects the
   dense part zx into PSUM. The dense matmuls zx = Wx^T·xin for the next
   chunk are spread between steps.
 - Roles are data-driven, not branch-driven: odd cores get zeroed x input
   and recv-mask 1 (evens 0), so one program computes both layers.
 - Host handles gate-column permutation to [f,i,j,o], direction reversal
   by lengths, final masking, and the 2-chunk output lag of odd cores.
"""

import numpy as np
from contextlib import ExitStack

import concourse.bass as bass
import concourse.bacc as bacc
import concourse.tile as tile
import concourse.mybir as mybir
from concourse.bass import ds, ts
from concourse.bass_utils import run_bass_kernel_spmd

F16 = mybir.dt.float16
F32 = mybir.dt.float32
AF = mybir.ActivationFunctionType

B, D, H, L = 32, 512, 512, 2
G = 4 * H            # 2048 gate rows
KT = H // 128        # 4 k-tiles
MT = G // 128        # 16 m-tiles
FORGET_BIAS = 1.0
GROUPS = [[0, 1], [2, 3], [4, 5], [6, 7]]


def build_program(T=1024, Tc=32, b=16, n_cores=8):
    """Uniform per-core program; returns compiled nc.

    Tick pipeline (tick i, parity p=i&1):
      chain(i): Tc recurrent steps on zx[p] -> st[p]
      mid/end of chain: masked half-chunk sends -> AllToAll
      recv(i): read AllToAll outputs of tick i-1 -> xin[1-p] += recv
      dense(i): zx[1-p] = Wx^T xin[1-p] (+FB on f gate), spread between steps
      y out: st[p] -> yT slot i
    """
    NCH = T // Tc
    NT = NCH + 2                   # 2 ticks of layer-1 lag
    Tc2 = Tc // 2                  # half-chunk send granularity
    CH = Tc * KT * b               # chunk free elems per partition
    CH2 = Tc2 * KT * b
    assert T % Tc == 0 and Tc % 2 == 0

    nc = bacc.Bacc("TRN2", target_bir_lowering=False, debug=False,
                   num_devices=n_cores)

    xT_d = nc.dram_tensor("xT", [128, (NT + 1) * Tc, KT, b], F16,
                          kind="ExternalInput")
    wx_d = nc.dram_tensor("wx", [KT, 128, G], F16, kind="ExternalInput")
    wh_d = nc.dram_tensor("wh", [KT, 128, G], F16, kind="ExternalInput")
    id_d = nc.dram_tensor("ident", [128, 128], F16, kind="ExternalInput")
    mask_d = nc.dram_tensor("rmask", [128, 1], F32, kind="ExternalInput")
    yT_d = nc.dram_tensor("yT", [128, NT * Tc, KT, b], F16,
                          kind="ExternalOutput")

    with tile.TileContext(nc) as tc, ExitStack() as ctx:
        wpool = ctx.enter_context(tc.tile_pool(name="w", bufs=1))
        pers = ctx.enter_context(tc.tile_pool(name="pers", bufs=1))
        gates = ctx.enter_context(tc.tile_pool(name="gates", bufs=3))
        psG = ctx.enter_context(tc.tile_pool(name="psG", bufs=1, space="PSUM"))
        psX = ctx.enter_context(tc.tile_pool(name="psX", bufs=4, space="PSUM"))
        dram = ctx.enter_context(tc.tile_pool(name="dram", bufs=1,
                                              space="DRAM"))

        wx_sb = wpool.tile([128, KT, G], F16, tag="wx", name="wx")
        wh_sb = wpool.tile([128, KT, G], F16, tag="wh", name="wh")
        ident = wpool.tile([128, 128], F16, tag="ident", name="ident")
        rmask = wpool.tile([128, 1], F32, tag="rmask", name="rmask")
        nc.sync.dma_start(out=ident[:], in_=id_d[:])
        nc.sync.dma_start(out=rmask[:], in_=mask_d[:])
        nc.sync.dma_start(out=wx_sb[:], in_=wx_d.rearrange("k p g -> p k g"))
        nc.sync.dma_start(out=wh_sb[:], in_=wh_d.rearrange("k p g -> p k g"))

        # persistent staging, double-buffered by tick parity
        zxP = [pers.tile([128, Tc, MT, b], F16, tag=f"zx{p}", name=f"zx{p}")
               for p in range(2)]
        xinP = [pers.tile([128, Tc, KT, b], F16, tag=f"xin{p}", name=f"xin{p}")
                for p in range(2)]
        stP = [pers.tile([128, Tc, KT, b], F16, tag=f"st{p}", name=f"st{p}")
               for p in range(2)]
        rvP = [[pers.tile([128, Tc2, KT, b], F16, tag=f"rv{p}{h}",
                          name=f"rv{p}{h}") for h in range(2)]
               for p in range(2)]
        cT = pers.tile([128, KT, b], F32, tag="cT", name="cT")

        # DRAM bounce buffers (AllGather: in = own half-chunk, out = 2 slots)
        binP = [[dram.tile([128, Tc2, KT, b], F16, tag=f"bin{p}{h}",
                           name=f"bin{p}{h}") for h in range(2)]
                for p in range(2)]
        boutP = [[dram.tile([2, 128, Tc2, KT, b], F16, tag=f"bout{p}{h}",
                            name=f"bout{p}{h}") for h in range(2)]
                 for p in range(2)]

        for p in range(2):
            nc.gpsimd.memset(zxP[p][:], 0.0)
            nc.gpsimd.memset(stP[p][:], 0.0)
            for h in range(2):
                nc.gpsimd.memset(rvP[p][h][:], 0.0)
        nc.gpsimd.memset(cT[:], 0.0)

        def xdma(p, t0):
            """Load x chunk starting at step t0 into xinP[p]."""
            nc.sync.dma_start(out=xinP[p][:], in_=xT_d[:, ds(t0, Tc), :, :])

        def send_half(p, h):
            """Half-chunk of stP[p] -> bounce -> AllGather with the pair.

            out slot 0 = even core's (layer-0) contribution; the odd core
            adds rmask(=1)*slot0 into its dense input, even cores rmask=0.
            """
            nc.gpsimd.dma_start(
                binP[p][h][:], stP[p][:, ds(h * Tc2, Tc2), :, :])
            nc.gpsimd.collective_compute(
                "AllGather", mybir.AluOpType.bypass,
                replica_groups=GROUPS,
                ins=[binP[p][h][:].opt()],
                outs=[boutP[p][h][:].opt()])

        def recv_half(p, h, into):
            """xin[half h] += rmask * (AllGather slot 0 of tick parity p).

            Runs on GpSimd (SBUF-only) so the DVE queue stays clear for the
            per-step chain ops.
            """
            nc.sync.dma_start(out=rvP[p][h][:], in_=boutP[p][h][0])
            nc.gpsimd.tensor_scalar_mul(rvP[p][h][:], rvP[p][h][:],
                                        rmask[:, 0:1])
            dst = xinP[into][:, ds(h * Tc2, Tc2), :, :]
            nc.gpsimd.tensor_add(dst, dst, rvP[p][h][:])

        Tc4 = Tc2 // 2

        def dense_unit(m, qh, into):
            """zx[into][:, quarter qh, m, :] = sum_k wx_k^T xin (+FB if f)."""
            ps = psX.tile([128, Tc4 * b], F32, tag="psx", name="psx")
            for k in range(KT):
                nc.tensor.matmul(
                    ps[:],
                    lhsT=wx_sb[:, k, m * 128:(m + 1) * 128],
                    rhs=xinP[into][:, ds(qh * Tc4, Tc4), k, :],
                    start=(k == 0), stop=(k == KT - 1))
            dst = zxP[into][:, ds(qh * Tc4, Tc4), m, :]
            psv = ps[:].rearrange("p (t b) -> p t b", t=Tc4)
            if m < 4:
                nc.vector.tensor_scalar_add(dst, psv, FORGET_BIAS)
            else:
                nc.vector.tensor_copy(dst, psv)

        def step(tl, p):
            """One recurrent step tl within tick of parity p.

            Each gate group gets its OWN PSUM tile so the per-gate ACT read
            never creates a (tile-granular) WAR against the next gate's
            matmul writes — that false dependency serializes the step.
            """
            pzg = [psG.tile([128, KT, b], F32, tag=f"pz{g}",
                            name=f"pz{g}") for g in range(4)]
            if tl == 0:
                hsrc = lambda k: stP[1 - p][:, Tc - 1, k, :]
            else:
                hsrc = lambda k: stP[p][:, tl - 1, k, :]
            # gate blocks: 0-3=f, 4-7=i, 8-11=j, 12-15=o
            gf = gates.tile([128, KT, b], F32, tag="gf", name="gf")
            gi = gates.tile([128, KT, b], F32, tag="gi", name="gi")
            tj = gates.tile([128, KT, b], F32, tag="tj", name="tj")
            go = gates.tile([128, KT, b], F16, tag="go", name="go")
            tch = gates.tile([128, KT, b], F16, tag="tch", name="tch")
            t1 = gates.tile([128, KT, b], F32, tag="t1", name="t1")
            t2 = gates.tile([128, KT, b], F32, tag="t2", name="t2")

            def mm_gate(g):
                nc.tensor.matmul(pzg[g][:], lhsT=ident[:],
                                 rhs=zxP[p][:, tl, ds(g * KT, KT), :],
                                 start=True, stop=False)
                for kk in range(KT):
                    m = g * KT + kk
                    for k in range(KT):
                        nc.tensor.matmul(
                            pzg[g][:, kk, :],
                            lhsT=wh_sb[:, k, m * 128:(m + 1) * 128],
                            rhs=hsrc(k),
                            start=False, stop=(k == KT - 1))

            mm_gate(0)                                       # f
            nc.scalar.activation(gf[:], pzg[0][:], AF.Sigmoid)
            mm_gate(1)                                       # i
            nc.scalar.activation(gi[:], pzg[1][:], AF.Sigmoid)
            nc.vector.tensor_mul(t1[:], gf[:], cT[:])
            mm_gate(2)                                       # j
            nc.scalar.activation(tj[:], pzg[2][:], AF.Tanh)
            nc.vector.tensor_mul(t2[:], gi[:], tj[:])
            mm_gate(3)                                       # o
            nc.vector.tensor_add(cT[:], t1[:], t2[:])
            nc.scalar.activation(go[:], pzg[3][:], AF.Sigmoid)
            nc.scalar.activation(tch[:], cT[:], AF.Tanh)
            nc.vector.tensor_mul(stP[p][:, tl, :, :], go[:], tch[:])

        def tick(i_reg, p, do_recv=True, do_send=True, do_dense=True,
                 x_t0=None):
            """One tick. i_reg: register/int of tick*Tc for DRAM addressing."""
            # prefetch next x chunk into xin (overwrites), before recv adds
            if do_dense:
                assert x_t0 is not None
                xdma(1 - p, x_t0)
            # dense units (quarter-chunks): recv h0 gates quarters 0-1,
            # recv h1 gates quarters 2-3; exactly 2 units per step.
            units = []
            if do_dense:
                units = [(m, q) for q in (0, 1) for m in range(MT)] + \
                        [(m, q) for q in (2, 3) for m in range(MT)]
            done = 0
            for tl in range(Tc):
                if do_recv and tl == 1:
                    recv_half(1 - p, 0, 1 - p)
                if do_recv and tl == Tc2 - 1:
                    recv_half(1 - p, 1, 1 - p)
                step(tl, p)
                if do_send and tl == Tc2:
                    send_half(p, 0)
                # spread: quarters 0-1 over steps [2, 17], 2-3 over [16, 31]
                if do_dense:
                    want = min(2 * MT, max(0, (tl - 1) * 2 * MT // (Tc2 - 1)))
                    want += max(0, (tl - Tc2 + 1) * 2 * MT // (Tc2 - 1))
                    want = min(want, len(units))
                    while done < want:
                        m, q = units[done]
                        dense_unit(m, q, 1 - p)
                        done += 1
            while done < len(units):
                m, q = units[done]
                dense_unit(m, q, 1 - p)
                done += 1
            if do_send:
                send_half(p, 1)
            nc.sync.dma_start(out=yT_d[:, ds(i_reg, Tc), :, :], in_=stP[p][:])

        # ---- peel: pre-tick dense for tick 0 (x chunk 0, no recv) ----
        xdma(0, 0)
        for q in range(4):
            for m in range(MT):
                dense_unit(m, q, 0)

        # tick 0: no recv (no prior CC); tick 1: full
        tick(0, 0, do_recv=False, x_t0=Tc)
        tick(Tc, 1, x_t0=2 * Tc)

        # ---- steady state: ticks 2..NT-3, fully unrolled ----
        for i in range(2, NT - 2):
            tick(i * Tc, i & 1, x_t0=(i + 1) * Tc)

        # ---- drain: tick NT-2 (recv+dense, no send), tick NT-1 (chain+y) --
        tick((NT - 2) * Tc, 0, do_send=False, x_t0=(NT - 1) * Tc)
        tick((NT - 1) * Tc, 1, do_recv=False, do_send=False, do_dense=False)

    nc.compile()
    return nc


# ---------------- host glue ----------------

def reverse_seq(x, lengths):
    t = np.arange(x.shape[1])[None, :]
    ln = lengths[:, None]
    idx = np.where(t < ln, ln - 1 - t, t)
    return np.take_along_axis(x, idx[:, :, None], axis=1)


def permute_gates(W):
    """[.., 4H] gate columns i,j,f,o -> f,i,j,o."""
    Wi, Wj, Wf, Wo = (W[..., 0:H], W[..., H:2 * H],
                      W[..., 2 * H:3 * H], W[..., 3 * H:4 * H])
    return np.concatenate([Wf, Wi, Wj, Wo], axis=-1)


def make_in_maps(inputs, lengths, Wf, Wb, T, Tc, b, n_cores=8):
    """Per-core inputs. Pair 2i/2i+1: even=L0, odd=L1."""
    NCH = T // Tc
    NT = NCH + 2
    xr = reverse_seq(inputs, lengths)
    in_maps = []
    ident = np.eye(128, dtype=np.float16)
    for c in range(n_cores):
        pair, role = c // 2, c % 2
        d, half = pair // 2, pair % 2
        bsel = slice(half * b, (half + 1) * b)
        W = permute_gates(np.asarray(Wf if d == 0 else Wb))[role]  # [1024,4H]
        wx = W[:D].reshape(KT, 128, G).astype(np.float16)
        wh = W[D:].reshape(KT, 128, G).astype(np.float16)
        if role == 0:
            x = (inputs if d == 0 else xr)[bsel, :T]      # [b, T, D]
            xT = x.transpose(2, 1, 0).reshape(KT, 128, T, b)
            xT = np.ascontiguousarray(xT.transpose(1, 2, 0, 3))  # [128,T,KT,b]
            xT = np.concatenate(
                [xT, np.zeros((128, (NT + 1) * Tc - T, KT, b), np.float16)],
                axis=1).astype(np.float16)
        else:
            xT = np.zeros((128, (NT + 1) * Tc, KT, b), np.float16)
        rmask = np.full((128, 1), float(role), np.float32)
        in_maps.append({"xT": xT, "wx": wx, "wh": wh, "ident": ident,
                        "rmask": rmask})
    return in_maps


def assemble_output(results, lengths, T, Tc, b, n_cores=8):
    """Odd cores' yT slots 2..NT-1 are the layer-1 output chunks 0..NCH-1."""
    out = np.zeros((B, T, 2 * H), np.float32)
    for c in range(1, n_cores, 2):
        pair = c // 2
        d, half = pair // 2, pair % 2
        s = half * b
        yT = results[c]["yT"].astype(np.float32)   # [128, NT*Tc, KT, b]
        yT = yT[:, 2 * Tc: 2 * Tc + T]             # un-lag
        y = yT.transpose(3, 1, 2, 0).reshape(b, T, H)
        if d == 0:
            out[s:s + b, :, :H] = y
        else:
            out[s:s + b, :, H:] = reverse_seq(y, lengths[s:s + b])
    mask = (np.arange(T)[None, :] < lengths[:, None])[:, :, None]
    return np.where(mask, out, 0.0).astype(np.float32)


# ---------------- grading entry point ----------------

_NC_CACHE = {}


def kernel(inputs, lengths, Wf, bf, Wb, bb):
    """Full-input BiLSTM encoder on 8 TRN2 NeuronCores.

    inputs: [32,1024,512] f32; lengths: [32] int; Wf/Wb: [2,1024,2048] f32;
    bf/bb: [2,2048] f32 (zeros; fixed FORGET_BIAS applied on-device).
    Returns [32,1024,1024] f32.
    """
    T, Tc, b = 1024, 32, 16
    inputs = np.asarray(inputs, dtype=np.float32)
    lengths = np.asarray(lengths).astype(np.int64)
    Wf = np.asarray(Wf, dtype=np.float32)
    Wb = np.asarray(Wb, dtype=np.float32)

    key = (T, Tc, b)
    if key not in _NC_CACHE:
        _NC_CACHE[key] = build_program(T=T, Tc=Tc, b=b)
    nc = _NC_CACHE[key]

    in_maps = make_in_maps(inputs, lengths, Wf, Wb, T, Tc, b)
    for _attempt in range(3):
        r = run_bass_kernel_spmd(nc, in_maps, list(range(8)), trace=False)
        out = assemble_output(r.results, lengths, T, Tc, b)
        if np.isfinite(out).all():
            return out
    return out


# revision 18
# speedup vs baseline: 1.2012x; 1.2012x over previous
"""BiLSTM encoder Bass/Tile kernel for TRN2 — layer-split across core pairs.

Architecture (8 cores, uniform SPMD program, one LSTM chain per core):
 - 4 core pairs: (0,1)=fwd batch 0:16, (2,3)=fwd 16:32, (4,5)=bwd 0:16,
   (6,7)=bwd 16:32. Even core runs layer 0, odd core runs layer 1 lagged by
   2 chunks; the layer-0 hidden-state chunks flow even->odd through an
   AllGather pair exchange (DRAM bounce buffers).
 - Each core runs ONE recurrent chain of T steps at b=16: per step 64
   LDW+MM pairs (N=16) for Wh·h plus one identity matmul that injects the
   dense part zx into PSUM. The dense matmuls zx = Wx^T·xin for the next
   chunk are spread between steps.
 - Roles are data-driven, not branch-driven: odd cores get zeroed x input
   and recv-mask 1 (evens 0), so one program computes both layers.
 - Host handles gate-column permutation to [f,i,j,o], direction reversal
   by lengths, final masking, and the 2-chunk output lag of odd cores.
"""

import numpy as np
from contextlib import ExitStack

import concourse.bass as bass
import concourse.bacc as bacc
import concourse.tile as tile
import concourse.mybir as mybir
from concourse.bass import ds, ts
from concourse.bass_utils import run_bass_kernel_spmd

F16 = mybir.dt.float16
F32 = mybir.dt.float32
AF = mybir.ActivationFunctionType

B, D, H, L = 32, 512, 512, 2
G = 4 * H            # 2048 gate rows
KT = H // 128        # 4 k-tiles
MT = G // 128        # 16 m-tiles
FORGET_BIAS = 1.0
GROUPS = [[0, 1], [2, 3], [4, 5], [6, 7]]


def build_program(T=1024, Tc=32, b=16, n_cores=8):
    """Uniform per-core program; returns compiled nc.

    Tick pipeline (tick i, parity p=i&1):
      chain(i): Tc recurrent steps on zx[p] -> st[p]
      mid/end of chain: masked half-chunk sends -> AllToAll
      recv(i): read AllToAll outputs of tick i-1 -> xin[1-p] += recv
      dense(i): zx[1-p] = Wx^T xin[1-p] (+FB on f gate), spread between steps
      y out: st[p] -> yT slot i
    """
    NCH = T // Tc
    NT = NCH + 2                   # 2 ticks of layer-1 lag
    Tc2 = Tc // 2                  # half-chunk send granularity
    CH = Tc * KT * b               # chunk free elems per partition
    CH2 = Tc2 * KT * b
    assert T % Tc == 0 and Tc % 2 == 0

    nc = bacc.Bacc("TRN2", target_bir_lowering=False, debug=False,
                   num_devices=n_cores)

    xT_d = nc.dram_tensor("xT", [128, (NT + 1) * Tc, KT, b], F16,
                          kind="ExternalInput")
    wx_d = nc.dram_tensor("wx", [KT, 128, G], F16, kind="ExternalInput")
    wh_d = nc.dram_tensor("wh", [KT, 128, G], F16, kind="ExternalInput")
    id_d = nc.dram_tensor("ident", [128, 128], F16, kind="ExternalInput")
    mask_d = nc.dram_tensor("rmask", [128, 1], F32, kind="ExternalInput")
    yT_d = nc.dram_tensor("yT", [128, NT * Tc, KT, b], F16,
                          kind="ExternalOutput")

    with tile.TileContext(nc) as tc, ExitStack() as ctx:
        wpool = ctx.enter_context(tc.tile_pool(name="w", bufs=1))
        pers = ctx.enter_context(tc.tile_pool(name="pers", bufs=1))
        gates = ctx.enter_context(tc.tile_pool(name="gates", bufs=3))
        psG = ctx.enter_context(tc.tile_pool(name="psG", bufs=1, space="PSUM"))
        psX = ctx.enter_context(tc.tile_pool(name="psX", bufs=4, space="PSUM"))
        dram = ctx.enter_context(tc.tile_pool(name="dram", bufs=1,
                                              space="DRAM"))

        wx_sb = wpool.tile([128, KT, G], F16, tag="wx", name="wx")
        wh_sb = wpool.tile([128, KT, G], F16, tag="wh", name="wh")
        ident = wpool.tile([128, 128], F16, tag="ident", name="ident")
        rmask = wpool.tile([128, 1], F32, tag="rmask", name="rmask")
        nc.sync.dma_start(out=ident[:], in_=id_d[:])
        nc.sync.dma_start(out=rmask[:], in_=mask_d[:])
        nc.sync.dma_start(out=wx_sb[:], in_=wx_d.rearrange("k p g -> p k g"))
        nc.sync.dma_start(out=wh_sb[:], in_=wh_d.rearrange("k p g -> p k g"))

        # persistent staging, double-buffered by tick parity
        zxP = [pers.tile([128, Tc, MT, b], F16, tag=f"zx{p}", name=f"zx{p}")
               for p in range(2)]
        # per-half xin staging: decouples dense-unit dependencies so the
        # dense matmuls don't all serialize behind the last recv add
        xinH = [[pers.tile([128, Tc2, KT, b], F16, tag=f"xin{p}{h}",
                           name=f"xin{p}{h}") for h in range(2)]
                for p in range(2)]
        stP = [pers.tile([128, Tc, KT, b], F16, tag=f"st{p}", name=f"st{p}")
               for p in range(2)]
        rvP = [[pers.tile([128, Tc2, KT, b], F16, tag=f"rv{p}{h}",
                          name=f"rv{p}{h}") for h in range(2)]
               for p in range(2)]
        cT = pers.tile([128, KT, b], F32, tag="cT", name="cT")

        # DRAM bounce buffers (AllGather: in = own half-chunk, out = 2 slots)
        binP = [[dram.tile([128, Tc2, KT, b], F16, tag=f"bin{p}{h}",
                           name=f"bin{p}{h}") for h in range(2)]
                for p in range(2)]
        boutP = [[dram.tile([2, 128, Tc2, KT, b], F16, tag=f"bout{p}{h}",
                            name=f"bout{p}{h}") for h in range(2)]
                 for p in range(2)]

        for p in range(2):
            nc.gpsimd.memset(zxP[p][:], 0.0)
            nc.gpsimd.memset(stP[p][:], 0.0)
            for h in range(2):
                nc.gpsimd.memset(rvP[p][h][:], 0.0)
        nc.gpsimd.memset(cT[:], 0.0)

        def xdma(p, t0):
            """Load x chunk starting at step t0 into xinH[p]."""
            for h in range(2):
                nc.sync.dma_start(
                    out=xinH[p][h][:],
                    in_=xT_d[:, ds(t0 + h * Tc2, Tc2), :, :])

        def send_half(p, h):
            """Half-chunk of stP[p] -> bounce -> AllGather with the pair.

            out slot 0 = even core's (layer-0) contribution; the odd core
            adds rmask(=1)*slot0 into its dense input, even cores rmask=0.
            """
            nc.gpsimd.dma_start(
                binP[p][h][:], stP[p][:, ds(h * Tc2, Tc2), :, :])
            nc.gpsimd.collective_compute(
                "AllGather", mybir.AluOpType.bypass,
                replica_groups=GROUPS,
                ins=[binP[p][h][:].opt()],
                outs=[boutP[p][h][:].opt()])

        def recv_dma(p, h):
            nc.sync.dma_start(out=rvP[p][h][:], in_=boutP[p][h][0])

        def recv_add(p, h, qq, into):
            """xin half h, quarter qq += rmask * recv (DVE, early in step)."""
            Tq = Tc2 // 2
            dst = xinH[into][h][:, ds(qq * Tq, Tq), :, :]
            nc.vector.scalar_tensor_tensor(
                dst, rvP[p][h][:, ds(qq * Tq, Tq), :, :], rmask[:, 0:1], dst,
                op0=mybir.AluOpType.mult, op1=mybir.AluOpType.add)

        Tc4 = Tc2 // 2

        def dense_unit(m, qh, into):
            """zx[into][:, quarter qh, m, :] = sum_k wx_k^T xin (+FB if f)."""
            ps = psX.tile([128, Tc4 * b], F32, tag="psx", name="psx")
            for k in range(KT):
                nc.tensor.matmul(
                    ps[:],
                    lhsT=wx_sb[:, k, m * 128:(m + 1) * 128],
                    rhs=xinH[into][qh // 2][:, ds((qh % 2) * Tc4, Tc4), k, :],
                    start=(k == 0), stop=(k == KT - 1))
            dst = zxP[into][:, ds(qh * Tc4, Tc4), m, :]
            psv = ps[:].rearrange("p (t b) -> p t b", t=Tc4)
            if m < 4:
                nc.vector.tensor_scalar_add(dst, psv, FORGET_BIAS)
            else:
                nc.vector.tensor_copy(dst, psv)

        def step(tl, p):
            """One recurrent step tl within tick of parity p.

            Each gate group gets its OWN PSUM tile so the per-gate ACT read
            never creates a (tile-granular) WAR against the next gate's
            matmul writes — that false dependency serializes the step.
            """
            pzg = [psG.tile([128, KT, b], F32, tag=f"pz{g}",
                            name=f"pz{g}") for g in range(4)]
            if tl == 0:
                hsrc = lambda k: stP[1 - p][:, Tc - 1, k, :]
            else:
                hsrc = lambda k: stP[p][:, tl - 1, k, :]
            # gate blocks: 0-3=f, 4-7=i, 8-11=j, 12-15=o
            gf = gates.tile([128, KT, b], F32, tag="gf", name="gf")
            gi = gates.tile([128, KT, b], F32, tag="gi", name="gi")
            tj = gates.tile([128, KT, b], F32, tag="tj", name="tj")
            go = gates.tile([128, KT, b], F16, tag="go", name="go")
            tch = gates.tile([128, KT, b], F16, tag="tch", name="tch")
            t1 = gates.tile([128, KT, b], F32, tag="t1", name="t1")
            t2 = gates.tile([128, KT, b], F32, tag="t2", name="t2")

            def mm_gate(g):
                nc.tensor.matmul(pzg[g][:], lhsT=ident[:],
                                 rhs=zxP[p][:, tl, ds(g * KT, KT), :],
                                 start=True, stop=False)
                for kk in range(KT):
                    m = g * KT + kk
                    for k in range(KT):
                        nc.tensor.matmul(
                            pzg[g][:, kk, :],
                            lhsT=wh_sb[:, k, m * 128:(m + 1) * 128],
                            rhs=hsrc(k),
                            start=False, stop=(k == KT - 1))

            mm_gate(0)                                       # f
            nc.scalar.activation(gf[:], pzg[0][:], AF.Sigmoid)
            mm_gate(1)                                       # i
            nc.scalar.activation(gi[:], pzg[1][:], AF.Sigmoid)
            nc.vector.tensor_mul(t1[:], gf[:], cT[:])
            mm_gate(2)                                       # j
            nc.scalar.activation(tj[:], pzg[2][:], AF.Tanh)
            nc.vector.tensor_mul(t2[:], gi[:], tj[:])
            mm_gate(3)                                       # o
            nc.vector.tensor_add(cT[:], t1[:], t2[:])
            nc.scalar.activation(go[:], pzg[3][:], AF.Sigmoid)
            nc.scalar.activation(tch[:], cT[:], AF.Tanh)
            nc.vector.tensor_mul(stP[p][:, tl, :, :], go[:], tch[:])

        def tick(i_reg, p, do_recv=True, do_send=True, do_dense=True,
                 do_xdma=True, x_t0=None):
            """One tick. i_reg: register/int of tick*Tc for DRAM addressing.

            The x prefetch issued here fills xinP[p] for the NEXT tick's
            dense (one full tick of DMA slack before its first reader).
            """
            if do_xdma:
                assert x_t0 is not None
                xdma(p, x_t0)
            # dense units (quarter-chunks): recv h0 gates quarters 0-1,
            # recv h1 gates quarters 2-3; exactly 2 units per step.
            units = []
            if do_dense:
                units = [(m, q) for q in (0, 1) for m in range(MT)] + \
                        [(m, q) for q in (2, 3) for m in range(MT)]
            done = 0
            for tl in range(Tc):
                if do_recv:
                    if tl == 0:
                        recv_dma(1 - p, 0)
                    elif tl in (1, 3):
                        recv_add(1 - p, 0, (tl - 1) // 2, 1 - p)
                    elif tl == 11:
                        recv_dma(1 - p, 1)
                    elif tl in (13, 15):
                        recv_add(1 - p, 1, (tl - 13) // 2, 1 - p)
                step(tl, p)
                if do_send and tl == Tc2:
                    send_half(p, 0)
                # spread: h0 units over steps [4, 19], h1 over [20, 31]
                if do_dense:
                    want = min(2 * MT, max(0, (tl - 3) * 2))
                    want += max(0, (tl - 19) * 2 * MT * 2 // 12)
                    want = min(want, len(units))
                    while done < want:
                        m, q = units[done]
                        dense_unit(m, q, 1 - p)
                        done += 1
            while done < len(units):
                m, q = units[done]
                dense_unit(m, q, 1 - p)
                done += 1
            if do_send:
                send_half(p, 1)
            nc.sync.dma_start(out=yT_d[:, ds(i_reg, Tc), :, :], in_=stP[p][:])

        # ---- peel: pre-tick dense for tick 0 (x chunks 0 and 1) ----
        xdma(0, 0)
        xdma(1, Tc)
        for q in range(4):
            for m in range(MT):
                dense_unit(m, q, 0)

        # tick 0: no recv (no prior CC); tick 1: full
        tick(0, 0, do_recv=False, x_t0=2 * Tc)
        tick(Tc, 1, x_t0=3 * Tc)

        # ---- steady state: ticks 2..NT-3, fully unrolled ----
        for i in range(2, NT - 2):
            tick(i * Tc, i & 1, x_t0=(i + 2) * Tc)

        # ---- drain: tick NT-2 (recv+dense, no send), tick NT-1 (chain+y) --
        tick((NT - 2) * Tc, 0, do_send=False, do_xdma=False)
        tick((NT - 1) * Tc, 1, do_recv=False, do_send=False, do_dense=False,
             do_xdma=False)

    nc.compile()
    return nc


# ---------------- host glue ----------------

def reverse_seq(x, lengths):
    t = np.arange(x.shape[1])[None, :]
    ln = lengths[:, None]
    idx = np.where(t < ln, ln - 1 - t, t)
    return np.take_along_axis(x, idx[:, :, None], axis=1)


def permute_gates(W):
    """[.., 4H] gate columns i,j,f,o -> f,i,j,o."""
    Wi, Wj, Wf, Wo = (W[..., 0:H], W[..., H:2 * H],
                      W[..., 2 * H:3 * H], W[..., 3 * H:4 * H])
    return np.concatenate([Wf, Wi, Wj, Wo], axis=-1)


def make_in_maps(inputs, lengths, Wf, Wb, T, Tc, b, n_cores=8):
    """Per-core inputs. Pair 2i/2i+1: even=L0, odd=L1."""
    NCH = T // Tc
    NT = NCH + 2
    xr = reverse_seq(inputs, lengths)
    in_maps = []
    ident = np.eye(128, dtype=np.float16)
    for c in range(n_cores):
        pair, role = c // 2, c % 2
        d, half = pair // 2, pair % 2
        bsel = slice(half * b, (half + 1) * b)
        W = permute_gates(np.asarray(Wf if d == 0 else Wb))[role]  # [1024,4H]
        wx = W[:D].reshape(KT, 128, G).astype(np.float16)
        wh = W[D:].reshape(KT, 128, G).astype(np.float16)
        if role == 0:
            x = (inputs if d == 0 else xr)[bsel, :T]      # [b, T, D]
            xT = x.transpose(2, 1, 0).reshape(KT, 128, T, b)
            xT = np.ascontiguousarray(xT.transpose(1, 2, 0, 3))  # [128,T,KT,b]
            xT = np.concatenate(
                [xT, np.zeros((128, (NT + 1) * Tc - T, KT, b), np.float16)],
                axis=1).astype(np.float16)
        else:
            xT = np.zeros((128, (NT + 1) * Tc, KT, b), np.float16)
        rmask = np.full((128, 1), float(role), np.float32)
        in_maps.append({"xT": xT, "wx": wx, "wh": wh, "ident": ident,
                        "rmask": rmask})
    return in_maps


def assemble_output(results, lengths, T, Tc, b, n_cores=8):
    """Odd cores' yT slots 2..NT-1 are the layer-1 output chunks 0..NCH-1."""
    out = np.zeros((B, T, 2 * H), np.float32)
    for c in range(1, n_cores, 2):
        pair = c // 2
        d, half = pair // 2, pair % 2
        s = half * b
        yT = results[c]["yT"].astype(np.float32)   # [128, NT*Tc, KT, b]
        yT = yT[:, 2 * Tc: 2 * Tc + T]             # un-lag
        y = yT.transpose(3, 1, 2, 0).reshape(b, T, H)
        if d == 0:
            out[s:s + b, :, :H] = y
        else:
            out[s:s + b, :, H:] = reverse_seq(y, lengths[s:s + b])
    mask = (np.arange(T)[None, :] < lengths[:, None])[:, :, None]
    return np.where(mask, out, 0.0).astype(np.float32)


# ---------------- grading entry point ----------------

_NC_CACHE = {}


def kernel(inputs, lengths, Wf, bf, Wb, bb):
    """Full-input BiLSTM encoder on 8 TRN2 NeuronCores.

    inputs: [32,1024,512] f32; lengths: [32] int; Wf/Wb: [2,1024,2048] f32;
    bf/bb: [2,2048] f32 (zeros; fixed FORGET_BIAS applied on-device).
    Returns [32,1024,1024] f32.
    """
    T, Tc, b = 1024, 32, 16
    inputs = np.asarray(inputs, dtype=np.float32)
    lengths = np.asarray(lengths).astype(np.int64)
    Wf = np.asarray(Wf, dtype=np.float32)
    Wb = np.asarray(Wb, dtype=np.float32)

    key = (T, Tc, b)
    if key not in _NC_CACHE:
        _NC_CACHE[key] = build_program(T=T, Tc=Tc, b=b)
    nc = _NC_CACHE[key]

    in_maps = make_in_maps(inputs, lengths, Wf, Wb, T, Tc, b)
    for _attempt in range(3):
        r = run_bass_kernel_spmd(nc, in_maps, list(range(8)), trace=False)
        out = assemble_output(r.results, lengths, T, Tc, b)
        if np.isfinite(out).all():
            return out
    return out
